# revision 23
# baseline (speedup 1.0000x reference)
"""Multi-head attention (B=2, S=2048, D=1024, H=16) on 8 TRN2 NeuronCores.

Sharding: DP=2 over batch x TP=4 over heads (4 heads/core). Per core:
QKV projections for its 256 output dims, attention for its 4 heads on its
batch, row-parallel output projection producing a partial [2048, 1024];
host sums the 4 partials per batch and adds bo (+ bv @ Wo.T, exact since
softmax weights sum to 1, so the v-bias never enters the device kernel).

Dataflow per core (all matmul operands bf16, fp32 PSUM accum):
  - x inputs host-packed to [4 sc, 128, 8 dc * 512] so each (input, sc)
    is ONE contiguous 1MB DMA; weights packed to [128, 8 dc * 256]
  - q/k projections -> per-chunk qh/kh tiles [128 dims, 512 seq] (bias
    fused into the PSUM->SBUF tensor_scalar_add)
  - v projection -> per-seq-tile vh [128 seq, 4*65] with an all-ones
    column per head (unnormalized attnV also yields the denominator)
  - scores transposed sT[k, q], two heads row-packed on the PE (K=64);
    exp on ACT (scale=1/8, no max subtraction: scores ~ N(0,1)) -> bf16
  - normalize via DVE reciprocal + gpsimd partition_broadcast + DVE mul
  - projection / out-projection matmuls are split into small quanta and
    emitted inside the attention j-loops (PE slack), with the first two
    j slots of each chunk kept clean so the exp pipeline never stalls
    at chunk boundaries; the previous chunk's normalization is emitted
    at slot j=1 (software pipelining)
"""
import numpy as np

B, S, D = 2, 2048, 1024
HEADS, DK = 16, 64
NCORES, DP, TP = 8, 2, 4
OPC = D // TP          # 256 output dims per core
HPC = HEADS // TP      # 4 heads per core
NDC = D // 128         # 8 contraction chunks
NST = S // 128         # 16 seq tiles
NSC = S // 512         # 4 seq chunks

_cache = {}


def _build():
    import concourse.mybir as mybir
    import concourse.tile as tile
    from concourse import bacc

    F32 = mybir.dt.float32
    BF16 = mybir.dt.bfloat16
    FP8 = mybir.dt.float8e4
    DR = mybir.MatmulPerfMode.DoubleRow
    Exp = mybir.ActivationFunctionType.Exp

    nc = bacc.Bacc("TRN2", target_bir_lowering=False, debug=False)

    xq_d = nc.dram_tensor("xqt", [NSC, 128, NDC * 512], BF16, kind="ExternalInput")
    xk_d = nc.dram_tensor("xkt", [NSC, 128, NDC * 512], BF16, kind="ExternalInput")
    xv_d = nc.dram_tensor("xvt", [NSC, 128, NDC * 512], BF16, kind="ExternalInput")
    wq_d = nc.dram_tensor("wqt", [128, NDC * OPC], BF16, kind="ExternalInput")
    wk_d = nc.dram_tensor("wkt", [128, NDC * OPC], BF16, kind="ExternalInput")
    wv_d = nc.dram_tensor("wvt", [128, NDC * OPC], BF16, kind="ExternalInput")
    bq_d = nc.dram_tensor("bq", [2, 128, 1], F32, kind="ExternalInput")
    bk_d = nc.dram_tensor("bk", [2, 128, 1], F32, kind="ExternalInput")
    wo_d = nc.dram_tensor("wot", [2, 128, D], BF16, kind="ExternalInput")
    out_d = nc.dram_tensor("out", [S, D], F32, kind="ExternalOutput")

    with tile.TileContext(nc) as tc:
        from contextlib import ExitStack
        es = ExitStack()
        with es:
            wp = es.enter_context(tc.tile_pool(name="wp", bufs=1))
            acts = es.enter_context(tc.tile_pool(name="acts", bufs=1))
            xp = es.enter_context(tc.tile_pool(name="xin", bufs=1))
            pps = es.enter_context(tc.tile_pool(name="pps", bufs=2, space="PSUM"))
            sps = es.enter_context(tc.tile_pool(name="sps", bufs=2, space="PSUM"))
            avps = es.enter_context(tc.tile_pool(name="avps", bufs=2, space="PSUM"))
            ep = es.enter_context(tc.tile_pool(name="ep", bufs=8))
            rp = es.enter_context(tc.tile_pool(name="rp", bufs=4))
            obp = es.enter_context(tc.tile_pool(name="obp", bufs=4))

            # persistent activations
            kh = [[acts.tile([128, 512], BF16, name=f"kh{hp}_{sc}")
                   for sc in range(NSC)] for hp in range(2)]
            qh = [[acts.tile([128, 512], BF16, name=f"qh{hp}_{sc}")
                   for sc in range(NSC)] for hp in range(2)]
            vh = [acts.tile([128, HPC * (DK + 1)], BF16, name=f"vh{st}")
                  for st in range(NST)]
            stacked = [[acts.tile([128, 512], BF16, name=f"st{hp}_{ic}")
                        for ic in range(NSC)] for hp in range(2)]

            # ones columns of vh first on the gpsimd queue
            for st in range(NST):
                nc.gpsimd.memset(vh[st][:], 1.0)

            # ---- DMAs, in need order, one per (tensor, chunk), all on sync
            wk_t = wp.tile([128, NDC * OPC], BF16, name="wk")
            wv_t = wp.tile([128, NDC * OPC], BF16, name="wv")
            wq_t = wp.tile([128, NDC * OPC], BF16, name="wq")
            xk_t = [None] * NSC
            xv_t = [None] * NSC
            xq_t = [None] * NSC

            def wsl(wt, dc, hp):
                return wt[:, dc * OPC + hp * 128: dc * OPC + (hp + 1) * 128]

            def xsl(xt, sc, dc, a, b):
                return xt[sc][:, dc * 512 + a: dc * 512 + b]

            def load_xc(xt, xd, tag, sc):
                if xt[sc] is None:
                    t = xp.tile([128, NDC * 512], BF16, name=f"{tag}{sc}")
                    nc.sync.dma_start(t[:], xd.ap()[sc])
                    xt[sc] = t

            nc.sync.dma_start(wk_t[:], wk_d.ap()[:, :])
            load_xc(xk_t, xk_d, "xk", 0)
            nc.sync.dma_start(wv_t[:], wv_d.ap()[:, :])
            load_xc(xv_t, xv_d, "xv", 0)
            nc.sync.dma_start(wq_t[:], wq_d.ap()[:, :])
            load_xc(xq_t, xq_d, "xq", 0)
            bq_t = [wp.tile([128, 1], F32, name=f"bq{h}") for h in range(2)]
            bk_t = [wp.tile([128, 1], F32, name=f"bk{h}") for h in range(2)]
            for h in range(2):
                nc.sync.dma_start(bq_t[h][:], bq_d.ap()[h])
                nc.sync.dma_start(bk_t[h][:], bk_d.ap()[h])
            for sc in (1, 2, 3):
                load_xc(xk_t, xk_d, "xk", sc)
                load_xc(xv_t, xv_d, "xv", sc)
            for sc in (1, 2, 3):
                load_xc(xq_t, xq_d, "xq", sc)
            wo_t = [wp.tile([128, D], BF16, name=f"wo{h}") for h in range(2)]
            for h in range(2):
                nc.sync.dma_start(wo_t[h][:], wo_d.ap()[h])

            def qk_proj_fillers(hp, sc, xt, wt, bias, dest):
                """One q/k projection chain as 4 PE quanta (2 MMs each)."""
                state = {}
                def mk(i):
                    def f():
                        if i == 0:
                            state["p"] = pps.tile([128, 512], F32,
                                                  name="pp", tag="pp")
                        p = state["p"]
                        for dc in (2 * i, 2 * i + 1):
                            nc.tensor.matmul(
                                p[:], wsl(wt, dc, hp), xsl(xt, sc, dc, 0, 512),
                                start=(dc == 0), stop=(dc == NDC - 1),
                                skip_group_check=True)
                        if i == 3:
                            nc.vector.tensor_scalar_add(
                                dest[hp][sc][:], p[:], bias[hp][:])
                    return f
                return [mk(i) for i in range(4)]

            def qk_proj(hp, sc, xt, wt, bias, dest):
                for f in qk_proj_fillers(hp, sc, xt, wt, bias, dest):
                    f()

            def v_proj(st):
                sc, half = divmod(st, 4)
                pv = pps.tile([128, OPC], F32, name="pp", tag="pp")
                for dc in range(NDC):
                    nc.tensor.matmul(
                        pv[:],
                        xsl(xv_t, sc, dc, half * 128, (half + 1) * 128),
                        wv_t[:, dc * OPC:(dc + 1) * OPC],
                        start=(dc == 0), stop=(dc == NDC - 1),
                        skip_group_check=True)
                dst = vh[st][:].rearrange("p (h x) -> p h x", h=HPC)[:, :, 0:DK]
                srcv = pv[:].rearrange("p (h d) -> p h d", h=HPC)
                nc.vector.tensor_copy(dst, srcv)

            def out_unit(ic, it4, mc, cp=None):
                po = pps.tile([128, 512], F32, name="pp", tag="pp")
                for hp in range(2):
                    nc.tensor.matmul(
                        po[:],
                        stacked[hp][ic][:, it4 * 128:(it4 + 1) * 128],
                        wo_t[hp][:, mc * 512:(mc + 1) * 512],
                        start=(hp == 0), stop=(hp == 1),
                        skip_group_check=True)
                ot = obp.tile([128, 512], F32, name="ot", tag="ot")
                if cp is None:
                    nc.vector.tensor_copy(ot[:], po[:])
                else:
                    cp(ot, po)
                it = ic * 4 + it4
                eng = nc.sync if mc == 0 else nc.gpsimd
                eng.dma_start(
                    out_d.ap()[it * 128:(it + 1) * 128,
                               mc * 512:(mc + 1) * 512], ot[:])

            def attn_chunk(hp, ic, fillers=(), per_j=1, pre=None,
                           fill_from=2):
                """Emit one attention chunk; returns a closure that emits
                its normalization (callers pass it as the NEXT chunk's
                `pre`, emitted at slot j=1 — software pipelining)."""
                fillers = list(fillers)
                av = [avps.tile([128, 512], F32, name="av", tag="av")
                      for _ in range(2)]
                ets = [None] * NST

                def emit_attnv(j):
                    for h2 in range(2):
                        h = hp * 2 + h2
                        nc.tensor.matmul(
                            av[h2][0:DK + 1, :],
                            vh[j][:, h * (DK + 1):(h + 1) * (DK + 1)],
                            ets[j][:, h2 * 512:(h2 + 1) * 512],
                            start=(j == 0), stop=(j == NST - 1),
                            skip_group_check=True)

                for j in range(NST):
                    sp = sps.tile([128, 1024], F32, name="sp", tag="sp")
                    ksc, kof = divmod(j, 4)
                    nc.tensor.matmul(
                        sp[:, 0:512],
                        kh[hp][ksc][0:64, kof * 128:(kof + 1) * 128],
                        qh[hp][ic][0:64, :],
                        start=True, stop=True, tile_position=(0, 0))
                    nc.tensor.matmul(
                        sp[:, 512:1024],
                        kh[hp][ksc][64:128, kof * 128:(kof + 1) * 128],
                        qh[hp][ic][64:128, :],
                        start=True, stop=True, tile_position=(64, 0))
                    ets[j] = ep.tile([128, 1024], BF16, name="et", tag="et")
                    nc.scalar.activation(ets[j][:], sp[:], Exp, scale=0.125)
                    if j == 1 and pre is not None:
                        pre()
                    if j >= 1:
                        emit_attnv(j - 1)   # lag-1: keeps scores ahead of ACT
                    if j >= fill_from and j < NST - 1:
                        for _ in range(per_j):
                            if fillers:
                                fillers.pop(0)()
                for f in fillers:
                    f()

                def finish():
                    emit_attnv(NST - 1)
                    dnm, rcf, r2s = [], [], []
                    for h2 in range(2):
                        dnm.append(rp.tile([1, 512], F32, name="dnm", tag="dnm"))
                        nc.vector.tensor_copy(dnm[h2][:], av[h2][DK:DK + 1, :])
                    for h2 in range(2):
                        rcf.append(rp.tile([1, 512], F32, name="rcf", tag="rcf"))
                        nc.vector.reciprocal_approx_fast(rcf[h2][:], dnm[h2][:])
                    for h2 in range(2):
                        r2s.append(rp.tile([64, 512], F32, name="r2s", tag="r2s"))
                        nc.gpsimd.partition_broadcast(r2s[h2][:], rcf[h2][:])
                    for h2 in range(2):
                        nc.vector.tensor_mul(
                            stacked[hp][ic][h2 * 64:(h2 + 1) * 64, :],
                            av[h2][0:DK, :], r2s[h2][:])
                return finish

            def qf(hp, sc, xt, wt, bias, dest):
                return qk_proj_fillers(hp, sc, xt, wt, bias, dest)

            def vf(st):
                return (lambda: v_proj(st))

            # ---- prologue: minimum work before chunk (0,0) can stream
            qk_proj(0, 0, xk_t, wk_t, bk_t, kh)
            for st in range(4):
                v_proj(st)
            qk_proj(0, 0, xq_t, wq_t, bq_t, qh)

            # chunk (0,0), 2 fillers/j from j0: kh[0][g] lands before j=4g,
            # vh[st] at least 2 slots before j=st; qh[0][1] at the end
            # deadline-safe + DMA-aware: qh[0][1] (xq0, already resident)
            # fills j0-1 while xk[1] is in flight; kh[0][g] quanta complete
            # at j=4g-5 (scores j=4g read them at iteration 4g); vh[st]
            # lands at iteration <= st (attnV(st) is emitted at st+1)
            f00 = qf(0, 1, xq_t, wq_t, bq_t, qh)
            f00 += qf(0, 1, xk_t, wk_t, bk_t, kh)
            f00 += [vf(st) for st in range(4, 8)]
            f00 += qf(0, 2, xk_t, wk_t, bk_t, kh)
            f00 += [vf(st) for st in range(8, 12)]
            f00 += qf(0, 3, xk_t, wk_t, bk_t, kh)
            f00 += [vf(st) for st in range(12, 16)]
            nrm = attn_chunk(0, 0, fillers=f00, per_j=2, fill_from=0)

            nrm = attn_chunk(0, 1, pre=nrm, fillers=(
                qf(0, 2, xq_t, wq_t, bq_t, qh)
                + qf(1, 0, xk_t, wk_t, bk_t, kh)
                + qf(1, 1, xk_t, wk_t, bk_t, kh)))
            nrm = attn_chunk(0, 2, pre=nrm, fillers=(
                qf(0, 3, xq_t, wq_t, bq_t, qh)
                + qf(1, 2, xk_t, wk_t, bk_t, kh)
                + qf(1, 3, xk_t, wk_t, bk_t, kh)))
            nrm = attn_chunk(0, 3, pre=nrm, fillers=(
                qf(1, 0, xq_t, wq_t, bq_t, qh)
                + qf(1, 1, xq_t, wq_t, bq_t, qh)
                + qf(1, 2, xq_t, wq_t, bq_t, qh)
                + qf(1, 3, xq_t, wq_t, bq_t, qh)))
            for ic in range(NSC):
                fill = []
                if ic > 0:
                    fill = [(lambda a, b, c: lambda: out_unit(a, b, c))
                            (ic - 1, it4, mc)
                            for it4 in range(4) for mc in range(2)]
                nrm = attn_chunk(1, ic, pre=nrm, fillers=fill)
            nrm()
            for it4 in range(4):
                for mc in range(2):
                    if (it4 + mc) % 2:
                        out_unit(3, it4, mc,
                                 cp=lambda o, p: nc.scalar.copy(o[:], p[:]))
                    else:
                        out_unit(3, it4, mc)

    nc.compile()
    return nc


def _prep_inputs(q, k, v, Wq, bq, Wk, bk, Wv, bv, Wo, bo):
    import ml_dtypes
    f = np.float32
    bf = ml_dtypes.bfloat16
    xT = {}
    for g in range(DP):
        for nm, a in (("q", q), ("k", k), ("v", v)):
            t = np.asarray(a[g], f).T.astype(bf)            # [1024, 2048]
            # [sc, p, dc*512+j] = t[dc*128+p, sc*512+j]
            t = t.reshape(NDC, 128, NSC, 512).transpose(2, 1, 0, 3)
            xT[(nm, g)] = np.ascontiguousarray(t.reshape(NSC, 128, NDC * 512))

    def packw(WT):   # [1024, 256] -> [128, 8*256]
        return np.ascontiguousarray(
            WT.reshape(NDC, 128, OPC).transpose(1, 0, 2).reshape(128, NDC * OPC))

    Wq, Wk, Wv, Wo = (np.asarray(a, f) for a in (Wq, Wk, Wv, Wo))
    bq, bk = (np.asarray(a, f) for a in (bq, bk))
    in_maps = []
    for c in range(NCORES):
        g, r = divmod(c, TP)
        sl = slice(r * OPC, (r + 1) * OPC)
        in_maps.append({
            "xqt": xT[("q", g)], "xkt": xT[("k", g)], "xvt": xT[("v", g)],
            "wqt": packw(Wq[sl].T.astype(bf)),
            "wkt": packw(Wk[sl].T.astype(bf)),
            "wvt": packw(Wv[sl].T.astype(bf)),
            "bq": bq[sl].reshape(2, 128, 1),
            "bk": bk[sl].reshape(2, 128, 1),
            "wot": np.ascontiguousarray(Wo[:, sl].T.astype(bf)).reshape(2, 128, D),
        })
    return in_maps


def kernel(q, k, v, Wq, bq, Wk, bk, Wv, bv, Wo, bo, _trace=False):
    from concourse.bass_utils import run_bass_kernel_spmd

    if "nc" not in _cache:
        _cache["nc"] = _build()
    nc = _cache["nc"]
    in_maps = _prep_inputs(q, k, v, Wq, bq, Wk, bk, Wv, bv, Wo, bo)
    res = run_bass_kernel_spmd(nc, in_maps, list(range(NCORES)), trace=_trace)
    _cache["last_exec_time_ns"] = res.exec_time_ns
    _cache["last_res"] = res
    parts = [res.results[c]["out"] for c in range(NCORES)]
    bo = np.asarray(bo, np.float32)
    bv = np.asarray(bv, np.float32)
    Wo = np.asarray(Wo, np.float32)
    bias = bo + bv @ Wo.T
    out = np.empty((B, S, D), np.float32)
    for g in range(DP):
        acc = parts[g * TP].astype(np.float32)
        for r in range(1, TP):
            acc = acc + parts[g * TP + r]
        out[g] = acc + bias
    return out


# revision 24
# speedup vs baseline: 1.0558x; 1.0558x over previous
"""Multi-head attention (B=2, S=2048, D=1024, H=16) on 8 TRN2 NeuronCores.

Sharding: DP=2 over batch x TP=4 over heads (4 heads/core). Per core:
QKV projections for its 256 output dims, attention for its 4 heads on its
batch, row-parallel output projection producing a partial [2048, 1024];
host sums the 4 partials per batch and adds bo (+ bv @ Wo.T, exact since
softmax weights sum to 1, so the v-bias never enters the device kernel).

Dataflow per core (all matmul operands bf16, fp32 PSUM accum):
  - x inputs host-packed to [4 sc, 128, 8 dc * 512] so each (input, sc)
    is ONE contiguous 1MB DMA; weights packed to [128, 8 dc * 256]
  - q/k projections -> per-chunk qh/kh tiles [128 dims, 512 seq] (bias
    fused into the PSUM->SBUF tensor_scalar_add)
  - v projection -> per-seq-tile vh [128 seq, 4*65] with an all-ones
    column per head (unnormalized attnV also yields the denominator)
  - scores transposed sT[k, q], two heads row-packed on the PE (K=64);
    exp on ACT (scale=1/8, no max subtraction: scores ~ N(0,1)) -> bf16
  - normalize via DVE reciprocal + gpsimd partition_broadcast + DVE mul
  - projection / out-projection matmuls are split into small quanta and
    emitted inside the attention j-loops (PE slack), with the first two
    j slots of each chunk kept clean so the exp pipeline never stalls
    at chunk boundaries; the previous chunk's normalization is emitted
    at slot j=1 (software pipelining)
"""
import numpy as np

B, S, D = 2, 2048, 1024
HEADS, DK = 16, 64
NCORES, DP, TP = 8, 2, 4
OPC = D // TP          # 256 output dims per core
HPC = HEADS // TP      # 4 heads per core
NDC = D // 128         # 8 contraction chunks
NST = S // 128         # 16 seq tiles
NSC = S // 512         # 4 seq chunks

_cache = {}


def _build():
    import concourse.mybir as mybir
    import concourse.tile as tile
    from concourse import bacc

    F32 = mybir.dt.float32
    BF16 = mybir.dt.bfloat16
    FP8 = mybir.dt.float8e4
    DR = mybir.MatmulPerfMode.DoubleRow
    Exp = mybir.ActivationFunctionType.Exp

    nc = bacc.Bacc("TRN2", target_bir_lowering=False, debug=False)

    xq_d = nc.dram_tensor("xqt", [NSC, 128, NDC * 512], BF16, kind="ExternalInput")
    xk_d = nc.dram_tensor("xkt", [NSC, 128, NDC * 512], BF16, kind="ExternalInput")
    xv_d = nc.dram_tensor("xvt", [NSC, 128, NDC * 512], BF16, kind="ExternalInput")
    wq_d = nc.dram_tensor("wqt", [128, NDC * OPC], BF16, kind="ExternalInput")
    wk_d = nc.dram_tensor("wkt", [128, NDC * OPC], BF16, kind="ExternalInput")
    wv_d = nc.dram_tensor("wvt", [128, NDC * OPC], BF16, kind="ExternalInput")
    bq_d = nc.dram_tensor("bq", [2, 128, 1], F32, kind="ExternalInput")
    bk_d = nc.dram_tensor("bk", [2, 128, 1], F32, kind="ExternalInput")
    wo_d = nc.dram_tensor("wot", [2, 128, D], BF16, kind="ExternalInput")
    out_d = nc.dram_tensor("out", [S, D], F32, kind="ExternalOutput")

    with tile.TileContext(nc) as tc:
        from contextlib import ExitStack
        es = ExitStack()
        with es:
            wp = es.enter_context(tc.tile_pool(name="wp", bufs=1))
            acts = es.enter_context(tc.tile_pool(name="acts", bufs=1))
            xp = es.enter_context(tc.tile_pool(name="xin", bufs=1))
            pps = es.enter_context(tc.tile_pool(name="pps", bufs=2, space="PSUM"))
            sps = es.enter_context(tc.tile_pool(name="sps", bufs=2, space="PSUM"))
            avps = es.enter_context(tc.tile_pool(name="avps", bufs=2, space="PSUM"))
            ep = es.enter_context(tc.tile_pool(name="ep", bufs=8))
            rp = es.enter_context(tc.tile_pool(name="rp", bufs=4))
            obp = es.enter_context(tc.tile_pool(name="obp", bufs=4))

            # persistent activations
            kh = [[acts.tile([128, 512], BF16, name=f"kh{hp}_{sc}")
                   for sc in range(NSC)] for hp in range(2)]
            qh = [[acts.tile([128, 512], BF16, name=f"qh{hp}_{sc}")
                   for sc in range(NSC)] for hp in range(2)]
            vh = [acts.tile([128, HPC * (DK + 1)], BF16, name=f"vh{st}")
                  for st in range(NST)]
            stacked = [[acts.tile([128, 512], BF16, name=f"st{hp}_{ic}")
                        for ic in range(NSC)] for hp in range(2)]

            # ones columns of vh first on the gpsimd queue
            for st in range(NST):
                nc.gpsimd.memset(vh[st][:], 1.0)

            # ---- DMAs, in need order, one per (tensor, chunk), all on sync
            wk_t = wp.tile([128, NDC * OPC], BF16, name="wk")
            wv_t = wp.tile([128, NDC * OPC], BF16, name="wv")
            wq_t = wp.tile([128, NDC * OPC], BF16, name="wq")
            xk_t = [None] * NSC
            xv_t = [None] * NSC
            xq_t = [None] * NSC

            def wsl(wt, dc, hp):
                return wt[:, dc * OPC + hp * 128: dc * OPC + (hp + 1) * 128]

            def xsl(xt, sc, dc, a, b):
                return xt[sc][:, dc * 512 + a: dc * 512 + b]

            def load_xc(xt, xd, tag, sc):
                if xt[sc] is None:
                    t = xp.tile([128, NDC * 512], BF16, name=f"{tag}{sc}")
                    nc.sync.dma_start(t[:], xd.ap()[sc])
                    xt[sc] = t

            nc.sync.dma_start(wk_t[:], wk_d.ap()[:, :])
            load_xc(xk_t, xk_d, "xk", 0)
            nc.sync.dma_start(wv_t[:], wv_d.ap()[:, :])
            load_xc(xv_t, xv_d, "xv", 0)
            nc.sync.dma_start(wq_t[:], wq_d.ap()[:, :])
            load_xc(xq_t, xq_d, "xq", 0)
            bq_t = [wp.tile([128, 1], F32, name=f"bq{h}") for h in range(2)]
            bk_t = [wp.tile([128, 1], F32, name=f"bk{h}") for h in range(2)]
            for h in range(2):
                nc.sync.dma_start(bq_t[h][:], bq_d.ap()[h])
                nc.sync.dma_start(bk_t[h][:], bk_d.ap()[h])
            load_xc(xk_t, xk_d, "xk", 1)
            load_xc(xv_t, xv_d, "xv", 1)
            load_xc(xq_t, xq_d, "xq", 1)
            for sc in (2, 3):
                load_xc(xk_t, xk_d, "xk", sc)
                load_xc(xv_t, xv_d, "xv", sc)
            for sc in (2, 3):
                load_xc(xq_t, xq_d, "xq", sc)
            wo_t = [wp.tile([128, D], BF16, name=f"wo{h}") for h in range(2)]
            for h in range(2):
                nc.sync.dma_start(wo_t[h][:], wo_d.ap()[h])

            def qk_proj_fillers(hp, sc, xt, wt, bias, dest):
                """One q/k projection chain as 4 PE quanta (2 MMs each)."""
                state = {}
                def mk(i):
                    def f():
                        if i == 0:
                            state["p"] = pps.tile([128, 512], F32,
                                                  name="pp", tag="pp")
                        p = state["p"]
                        for dc in (2 * i, 2 * i + 1):
                            nc.tensor.matmul(
                                p[:], wsl(wt, dc, hp), xsl(xt, sc, dc, 0, 512),
                                start=(dc == 0), stop=(dc == NDC - 1),
                                skip_group_check=True)
                        if i == 3:
                            nc.vector.tensor_scalar_add(
                                dest[hp][sc][:], p[:], bias[hp][:])
                    return f
                return [mk(i) for i in range(4)]

            def qk_proj(hp, sc, xt, wt, bias, dest):
                for f in qk_proj_fillers(hp, sc, xt, wt, bias, dest):
                    f()

            def v_proj(st):
                sc, half = divmod(st, 4)
                pv = pps.tile([128, OPC], F32, name="pp", tag="pp")
                for dc in range(NDC):
                    nc.tensor.matmul(
                        pv[:],
                        xsl(xv_t, sc, dc, half * 128, (half + 1) * 128),
                        wv_t[:, dc * OPC:(dc + 1) * OPC],
                        start=(dc == 0), stop=(dc == NDC - 1),
                        skip_group_check=True)
                dst = vh[st][:].rearrange("p (h x) -> p h x", h=HPC)[:, :, 0:DK]
                srcv = pv[:].rearrange("p (h d) -> p h d", h=HPC)
                nc.vector.tensor_copy(dst, srcv)

            def out_unit(ic, it4, mc, cp=None):
                po = pps.tile([128, 512], F32, name="pp", tag="pp")
                for hp in range(2):
                    nc.tensor.matmul(
                        po[:],
                        stacked[hp][ic][:, it4 * 128:(it4 + 1) * 128],
                        wo_t[hp][:, mc * 512:(mc + 1) * 512],
                        start=(hp == 0), stop=(hp == 1),
                        skip_group_check=True)
                ot = obp.tile([128, 512], F32, name="ot", tag="ot")
                if cp is None:
                    nc.vector.tensor_copy(ot[:], po[:])
                else:
                    cp(ot, po)
                it = ic * 4 + it4
                eng = nc.sync if mc == 0 else nc.gpsimd
                eng.dma_start(
                    out_d.ap()[it * 128:(it + 1) * 128,
                               mc * 512:(mc + 1) * 512], ot[:])

            def attn_chunk(hp, ic, fillers=(), per_j=1, pre=None,
                           fill_from=2):
                """Emit one attention chunk; returns a closure that emits
                its normalization (callers pass it as the NEXT chunk's
                `pre`, emitted at slot j=1 — software pipelining)."""
                fillers = list(fillers)
                av = [avps.tile([128, 512], F32, name="av", tag="av")
                      for _ in range(2)]
                ets = [None] * NST

                def emit_attnv(j):
                    for h2 in range(2):
                        h = hp * 2 + h2
                        nc.tensor.matmul(
                            av[h2][0:DK + 1, :],
                            vh[j][:, h * (DK + 1):(h + 1) * (DK + 1)],
                            ets[j][:, h2 * 512:(h2 + 1) * 512],
                            start=(j == 0), stop=(j == NST - 1),
                            skip_group_check=True)

                for j in range(NST):
                    sp = sps.tile([128, 1024], F32, name="sp", tag="sp")
                    ksc, kof = divmod(j, 4)
                    nc.tensor.matmul(
                        sp[:, 0:512],
                        kh[hp][ksc][0:64, kof * 128:(kof + 1) * 128],
                        qh[hp][ic][0:64, :],
                        start=True, stop=True, tile_position=(0, 0))
                    nc.tensor.matmul(
                        sp[:, 512:1024],
                        kh[hp][ksc][64:128, kof * 128:(kof + 1) * 128],
                        qh[hp][ic][64:128, :],
                        start=True, stop=True, tile_position=(64, 0))
                    ets[j] = ep.tile([128, 1024], BF16, name="et", tag="et")
                    nc.scalar.activation(ets[j][:], sp[:], Exp, scale=0.125)
                    if j == 1 and pre is not None:
                        pre()
                    if j >= 1:
                        emit_attnv(j - 1)   # lag-1: keeps scores ahead of ACT
                    if j >= fill_from and j < NST - 1:
                        for _ in range(per_j):
                            if fillers:
                                fillers.pop(0)()
                for f in fillers:
                    f()

                def finish():
                    emit_attnv(NST - 1)
                    dnm, rcf, r2s = [], [], []
                    for h2 in range(2):
                        dnm.append(rp.tile([1, 512], F32, name="dnm", tag="dnm"))
                        nc.vector.tensor_copy(dnm[h2][:], av[h2][DK:DK + 1, :])
                    for h2 in range(2):
                        rcf.append(rp.tile([1, 512], F32, name="rcf", tag="rcf"))
                        nc.vector.reciprocal_approx_fast(rcf[h2][:], dnm[h2][:])
                    for h2 in range(2):
                        r2s.append(rp.tile([64, 512], F32, name="r2s", tag="r2s"))
                        nc.gpsimd.partition_broadcast(r2s[h2][:], rcf[h2][:])
                    for h2 in range(2):
                        nc.vector.tensor_mul(
                            stacked[hp][ic][h2 * 64:(h2 + 1) * 64, :],
                            av[h2][0:DK, :], r2s[h2][:])
                return finish

            def qf(hp, sc, xt, wt, bias, dest):
                return qk_proj_fillers(hp, sc, xt, wt, bias, dest)

            def vf(st):
                return (lambda: v_proj(st))

            # ---- prologue: minimum work before chunk (0,0) can stream
            qk_proj(0, 0, xk_t, wk_t, bk_t, kh)
            for st in range(4):
                v_proj(st)
            qk_proj(0, 0, xq_t, wq_t, bq_t, qh)

            # chunk (0,0), 2 fillers/j from j0: kh[0][g] lands before j=4g,
            # vh[st] at least 2 slots before j=st; qh[0][1] at the end
            # fillers ordered by DMA arrival (== deadline order): kh[0][g]
            # completes by slot 2g+3 <= scores deadline j=4g; vh[st] lands
            # before attnV(st) (emitted at iteration st+1); qh[0][1] last
            # (xq[1] arrives mid-chunk). j0-1 kept clean: they only need
            # sc0 data, so the exp stream starts as early as possible.
            f00 = qf(0, 1, xk_t, wk_t, bk_t, kh)
            f00 += [vf(st) for st in range(4, 8)]
            f00 += qf(0, 2, xk_t, wk_t, bk_t, kh)
            f00 += [vf(st) for st in range(8, 12)]
            f00 += qf(0, 3, xk_t, wk_t, bk_t, kh)
            f00 += [vf(st) for st in range(12, 16)]
            f00 += qf(0, 1, xq_t, wq_t, bq_t, qh)
            nrm = attn_chunk(0, 0, fillers=f00, per_j=2, fill_from=2)

            nrm = attn_chunk(0, 1, pre=nrm, fillers=(
                qf(0, 2, xq_t, wq_t, bq_t, qh)
                + qf(1, 0, xk_t, wk_t, bk_t, kh)
                + qf(1, 1, xk_t, wk_t, bk_t, kh)))
            nrm = attn_chunk(0, 2, pre=nrm, fillers=(
                qf(0, 3, xq_t, wq_t, bq_t, qh)
                + qf(1, 2, xk_t, wk_t, bk_t, kh)
                + qf(1, 3, xk_t, wk_t, bk_t, kh)))
            nrm = attn_chunk(0, 3, pre=nrm, fillers=(
                qf(1, 0, xq_t, wq_t, bq_t, qh)
                + qf(1, 1, xq_t, wq_t, bq_t, qh)
                + qf(1, 2, xq_t, wq_t, bq_t, qh)
                + qf(1, 3, xq_t, wq_t, bq_t, qh)))
            for ic in range(NSC):
                fill = []
                if ic > 0:
                    fill = [(lambda a, b, c: lambda: out_unit(a, b, c))
                            (ic - 1, it4, mc)
                            for it4 in range(4) for mc in range(2)]
                nrm = attn_chunk(1, ic, pre=nrm, fillers=fill)
            nrm()
            for it4 in range(4):
                for mc in range(2):
                    if (it4 + mc) % 2:
                        out_unit(3, it4, mc,
                                 cp=lambda o, p: nc.scalar.copy(o[:], p[:]))
                    else:
                        out_unit(3, it4, mc)

    nc.compile()
    return nc


def _prep_inputs(q, k, v, Wq, bq, Wk, bk, Wv, bv, Wo, bo):
    import ml_dtypes
    f = np.float32
    bf = ml_dtypes.bfloat16
    xT = {}
    for g in range(DP):
        for nm, a in (("q", q), ("k", k), ("v", v)):
            t = np.asarray(a[g], f).T.astype(bf)            # [1024, 2048]
            # [sc, p, dc*512+j] = t[dc*128+p, sc*512+j]
            t = t.reshape(NDC, 128, NSC, 512).transpose(2, 1, 0, 3)
            xT[(nm, g)] = np.ascontiguousarray(t.reshape(NSC, 128, NDC * 512))

    def packw(WT):   # [1024, 256] -> [128, 8*256]
        return np.ascontiguousarray(
            WT.reshape(NDC, 128, OPC).transpose(1, 0, 2).reshape(128, NDC * OPC))

    Wq, Wk, Wv, Wo = (np.asarray(a, f) for a in (Wq, Wk, Wv, Wo))
    bq, bk = (np.asarray(a, f) for a in (bq, bk))
    in_maps = []
    for c in range(NCORES):
        g, r = divmod(c, TP)
        sl = slice(r * OPC, (r + 1) * OPC)
        in_maps.append({
            "xqt": xT[("q", g)], "xkt": xT[("k", g)], "xvt": xT[("v", g)],
            "wqt": packw(Wq[sl].T.astype(bf)),
            "wkt": packw(Wk[sl].T.astype(bf)),
            "wvt": packw(Wv[sl].T.astype(bf)),
            "bq": bq[sl].reshape(2, 128, 1),
            "bk": bk[sl].reshape(2, 128, 1),
            "wot": np.ascontiguousarray(Wo[:, sl].T.astype(bf)).reshape(2, 128, D),
        })
    return in_maps


def kernel(q, k, v, Wq, bq, Wk, bk, Wv, bv, Wo, bo, _trace=False):
    from concourse.bass_utils import run_bass_kernel_spmd

    if "nc" not in _cache:
        _cache["nc"] = _build()
    nc = _cache["nc"]
    in_maps = _prep_inputs(q, k, v, Wq, bq, Wk, bk, Wv, bv, Wo, bo)
    res = run_bass_kernel_spmd(nc, in_maps, list(range(NCORES)), trace=_trace)
    _cache["last_exec_time_ns"] = res.exec_time_ns
    _cache["last_res"] = res
    parts = [res.results[c]["out"] for c in range(NCORES)]
    bo = np.asarray(bo, np.float32)
    bv = np.asarray(bv, np.float32)
    Wo = np.asarray(Wo, np.float32)
    bias = bo + bv @ Wo.T
    out = np.empty((B, S, D), np.float32)
    for g in range(DP):
        acc = parts[g * TP].astype(np.float32)
        for r in range(1, TP):
            acc = acc + parts[g * TP + r]
        out[g] = acc + bias
    return out


# revision 25
# speedup vs baseline: 1.0645x; 1.0083x over previous
"""Multi-head attention (B=2, S=2048, D=1024, H=16) on 8 TRN2 NeuronCores.

Sharding: DP=2 over batch x TP=4 over heads (4 heads/core). Per core:
QKV projections for its 256 output dims, attention for its 4 heads on its
batch, row-parallel output projection producing a partial [2048, 1024];
host sums the 4 partials per batch and adds bo (+ bv @ Wo.T, exact since
softmax weights sum to 1, so the v-bias never enters the device kernel).

Dataflow per core (all matmul operands bf16, fp32 PSUM accum):
  - x inputs host-packed to [4 sc, 128, 8 dc * 512] so each (input, sc)
    is ONE contiguous 1MB DMA; weights packed to [128, 8 dc * 256]
  - q/k projections -> per-chunk qh/kh tiles [128 dims, 512 seq] (bias
    fused into the PSUM->SBUF tensor_scalar_add)
  - v projection -> per-seq-tile vh [128 seq, 4*65] with an all-ones
    column per head (unnormalized attnV also yields the denominator)
  - scores transposed sT[k, q], two heads row-packed on the PE (K=64);
    exp on ACT (scale=1/8, no max subtraction: scores ~ N(0,1)) -> bf16
  - normalize via DVE reciprocal + gpsimd partition_broadcast + DVE mul
  - projection / out-projection matmuls are split into small quanta and
    emitted inside the attention j-loops (PE slack), with the first two
    j slots of each chunk kept clean so the exp pipeline never stalls
    at chunk boundaries; the previous chunk's normalization is emitted
    at slot j=1 (software pipelining)
"""
import numpy as np

B, S, D = 2, 2048, 1024
HEADS, DK = 16, 64
NCORES, DP, TP = 8, 2, 4
OPC = D // TP          # 256 output dims per core
HPC = HEADS // TP      # 4 heads per core
NDC = D // 128         # 8 contraction chunks
NST = S // 128         # 16 seq tiles
NSC = S // 512         # 4 seq chunks

_cache = {}


def _build():
    import concourse.mybir as mybir
    import concourse.tile as tile
    from concourse import bacc

    F32 = mybir.dt.float32
    BF16 = mybir.dt.bfloat16
    FP8 = mybir.dt.float8e4
    DR = mybir.MatmulPerfMode.DoubleRow
    Exp = mybir.ActivationFunctionType.Exp

    nc = bacc.Bacc("TRN2", target_bir_lowering=False, debug=False)

    xq_d = nc.dram_tensor("xqt", [NSC, 128, NDC * 512], BF16, kind="ExternalInput")
    xk_d = nc.dram_tensor("xkt", [NSC, 128, NDC * 512], BF16, kind="ExternalInput")
    xv_d = nc.dram_tensor("xvt", [NSC, 128, NDC * 512], BF16, kind="ExternalInput")
    wq_d = nc.dram_tensor("wqt", [128, NDC * OPC], BF16, kind="ExternalInput")
    wk_d = nc.dram_tensor("wkt", [128, NDC * OPC], BF16, kind="ExternalInput")
    wv_d = nc.dram_tensor("wvt", [128, NDC * OPC], BF16, kind="ExternalInput")
    bq_d = nc.dram_tensor("bq", [2, 128, 1], F32, kind="ExternalInput")
    bk_d = nc.dram_tensor("bk", [2, 128, 1], F32, kind="ExternalInput")
    wo_d = nc.dram_tensor("wot", [2, 128, D], BF16, kind="ExternalInput")
    out_d = nc.dram_tensor("out", [S, D], F32, kind="ExternalOutput")

    with tile.TileContext(nc) as tc:
        from contextlib import ExitStack
        es = ExitStack()
        with es:
            wp = es.enter_context(tc.tile_pool(name="wp", bufs=1))
            acts = es.enter_context(tc.tile_pool(name="acts", bufs=1))
            xp = es.enter_context(tc.tile_pool(name="xin", bufs=1))
            pps = es.enter_context(tc.tile_pool(name="pps", bufs=2, space="PSUM"))
            sps = es.enter_context(tc.tile_pool(name="sps", bufs=2, space="PSUM"))
            avps = es.enter_context(tc.tile_pool(name="avps", bufs=2, space="PSUM"))
            ep = es.enter_context(tc.tile_pool(name="ep", bufs=8))
            rp = es.enter_context(tc.tile_pool(name="rp", bufs=4))
            obp = es.enter_context(tc.tile_pool(name="obp", bufs=4))

            # persistent activations
            kh = [[acts.tile([128, 512], BF16, name=f"kh{hp}_{sc}")
                   for sc in range(NSC)] for hp in range(2)]
            qh = [[acts.tile([128, 512], BF16, name=f"qh{hp}_{sc}")
                   for sc in range(NSC)] for hp in range(2)]
            vh = [acts.tile([128, HPC * (DK + 1)], BF16, name=f"vh{st}")
                  for st in range(NST)]
            stacked = [[acts.tile([128, 512], BF16, name=f"st{hp}_{ic}")
                        for ic in range(NSC)] for hp in range(2)]

            # ones columns of vh first on the gpsimd queue
            for st in range(NST):
                nc.gpsimd.memset(vh[st][:], 1.0)

            # ---- DMAs, in need order, one per (tensor, chunk), all on sync
            wk_t = wp.tile([128, NDC * OPC], BF16, name="wk")
            wv_t = wp.tile([128, NDC * OPC], BF16, name="wv")
            wq_t = wp.tile([128, NDC * OPC], BF16, name="wq")
            xk_t = [None] * NSC
            xv_t = [None] * NSC
            xq_t = [None] * NSC

            def wsl(wt, dc, hp):
                return wt[:, dc * OPC + hp * 128: dc * OPC + (hp + 1) * 128]

            def xsl(xt, sc, dc, a, b):
                return xt[sc][:, dc * 512 + a: dc * 512 + b]

            def load_xc(xt, xd, tag, sc):
                if xt[sc] is None:
                    t = xp.tile([128, NDC * 512], BF16, name=f"{tag}{sc}")
                    nc.sync.dma_start(t[:], xd.ap()[sc])
                    xt[sc] = t

            nc.sync.dma_start(wk_t[:], wk_d.ap()[:, :])
            load_xc(xk_t, xk_d, "xk", 0)
            nc.sync.dma_start(wv_t[:], wv_d.ap()[:, :])
            load_xc(xv_t, xv_d, "xv", 0)
            nc.sync.dma_start(wq_t[:], wq_d.ap()[:, :])
            load_xc(xq_t, xq_d, "xq", 0)
            bq_t = [wp.tile([128, 1], F32, name=f"bq{h}") for h in range(2)]
            bk_t = [wp.tile([128, 1], F32, name=f"bk{h}") for h in range(2)]
            for h in range(2):
                nc.sync.dma_start(bq_t[h][:], bq_d.ap()[h])
                nc.sync.dma_start(bk_t[h][:], bk_d.ap()[h])
            load_xc(xk_t, xk_d, "xk", 1)
            load_xc(xv_t, xv_d, "xv", 1)
            load_xc(xq_t, xq_d, "xq", 1)
            for sc in (2, 3):
                load_xc(xk_t, xk_d, "xk", sc)
                load_xc(xv_t, xv_d, "xv", sc)
            for sc in (2, 3):
                load_xc(xq_t, xq_d, "xq", sc)
            wo_t = [wp.tile([128, D], BF16, name=f"wo{h}") for h in range(2)]
            for h in range(2):
                nc.sync.dma_start(wo_t[h][:], wo_d.ap()[h])

            def qk_proj_fillers(hp, sc, xt, wt, bias, dest):
                """One q/k projection chain as 4 PE quanta (2 MMs each)."""
                state = {}
                def mk(i):
                    def f():
                        if i == 0:
                            state["p"] = pps.tile([128, 512], F32,
                                                  name="pp", tag="pp")
                        p = state["p"]
                        for dc in (2 * i, 2 * i + 1):
                            nc.tensor.matmul(
                                p[:], wsl(wt, dc, hp), xsl(xt, sc, dc, 0, 512),
                                start=(dc == 0), stop=(dc == NDC - 1),
                                skip_group_check=True)
                        if i == 3:
                            nc.vector.tensor_scalar_add(
                                dest[hp][sc][:], p[:], bias[hp][:])
                    return f
                return [mk(i) for i in range(4)]

            def qk_proj(hp, sc, xt, wt, bias, dest):
                for f in qk_proj_fillers(hp, sc, xt, wt, bias, dest):
                    f()

            def v_proj(st):
                sc, half = divmod(st, 4)
                pv = pps.tile([128, OPC], F32, name="pp", tag="pp")
                for dc in range(NDC):
                    nc.tensor.matmul(
                        pv[:],
                        xsl(xv_t, sc, dc, half * 128, (half + 1) * 128),
                        wv_t[:, dc * OPC:(dc + 1) * OPC],
                        start=(dc == 0), stop=(dc == NDC - 1),
                        skip_group_check=True)
                dst = vh[st][:].rearrange("p (h x) -> p h x", h=HPC)[:, :, 0:DK]
                srcv = pv[:].rearrange("p (h d) -> p h d", h=HPC)
                nc.vector.tensor_copy(dst, srcv)

            def out_unit(ic, it4, mc, cp=None):
                po = pps.tile([128, 512], F32, name="pp", tag="pp")
                for hp in range(2):
                    nc.tensor.matmul(
                        po[:],
                        stacked[hp][ic][:, it4 * 128:(it4 + 1) * 128],
                        wo_t[hp][:, mc * 512:(mc + 1) * 512],
                        start=(hp == 0), stop=(hp == 1),
                        skip_group_check=True)
                ot = obp.tile([128, 512], F32, name="ot", tag="ot")
                if cp is None:
                    nc.vector.tensor_copy(ot[:], po[:])
                else:
                    cp(ot, po)
                it = ic * 4 + it4
                nc.sync.dma_start(
                    out_d.ap()[it * 128:(it + 1) * 128,
                               mc * 512:(mc + 1) * 512], ot[:])

            def attn_chunk(hp, ic, fillers=(), per_j=1, pre=None,
                           fill_from=2):
                """Emit one attention chunk; returns a closure that emits
                its normalization (callers pass it as the NEXT chunk's
                `pre`, emitted at slot j=1 — software pipelining)."""
                fillers = list(fillers)
                av = [avps.tile([128, 512], F32, name="av", tag="av")
                      for _ in range(2)]
                ets = [None] * NST

                def emit_attnv(j):
                    for h2 in range(2):
                        h = hp * 2 + h2
                        nc.tensor.matmul(
                            av[h2][0:DK + 1, :],
                            vh[j][:, h * (DK + 1):(h + 1) * (DK + 1)],
                            ets[j][:, h2 * 512:(h2 + 1) * 512],
                            start=(j == 0), stop=(j == NST - 1),
                            skip_group_check=True)

                for j in range(NST):
                    sp = sps.tile([128, 1024], F32, name="sp", tag="sp")
                    ksc, kof = divmod(j, 4)
                    nc.tensor.matmul(
                        sp[:, 0:512],
                        kh[hp][ksc][0:64, kof * 128:(kof + 1) * 128],
                        qh[hp][ic][0:64, :],
                        start=True, stop=True, tile_position=(0, 0))
                    nc.tensor.matmul(
                        sp[:, 512:1024],
                        kh[hp][ksc][64:128, kof * 128:(kof + 1) * 128],
                        qh[hp][ic][64:128, :],
                        start=True, stop=True, tile_position=(64, 0))
                    ets[j] = ep.tile([128, 1024], BF16, name="et", tag="et")
                    nc.scalar.activation(ets[j][:], sp[:], Exp, scale=0.125)
                    if j == 1 and pre is not None:
                        pre()
                    if j >= 1:
                        emit_attnv(j - 1)   # lag-1: keeps scores ahead of ACT
                    if j >= fill_from and j < NST - 1:
                        for _ in range(per_j):
                            if fillers:
                                fillers.pop(0)()
                for f in fillers:
                    f()

                def finish():
                    emit_attnv(NST - 1)
                    dnm, rcf, r2s = [], [], []
                    for h2 in range(2):
                        dnm.append(rp.tile([1, 512], F32, name="dnm", tag="dnm"))
                        nc.vector.tensor_copy(dnm[h2][:], av[h2][DK:DK + 1, :])
                    for h2 in range(2):
                        rcf.append(rp.tile([1, 512], F32, name="rcf", tag="rcf"))
                        nc.vector.reciprocal_approx_fast(rcf[h2][:], dnm[h2][:])
                    for h2 in range(2):
                        r2s.append(rp.tile([64, 512], F32, name="r2s", tag="r2s"))
                        nc.gpsimd.partition_broadcast(r2s[h2][:], rcf[h2][:])
                    for h2 in range(2):
                        nc.vector.tensor_mul(
                            stacked[hp][ic][h2 * 64:(h2 + 1) * 64, :],
                            av[h2][0:DK, :], r2s[h2][:])
                return finish

            def qf(hp, sc, xt, wt, bias, dest):
                return qk_proj_fillers(hp, sc, xt, wt, bias, dest)

            def vf(st):
                return (lambda: v_proj(st))

            # ---- prologue: minimum work before chunk (0,0) can stream
            qk_proj(0, 0, xk_t, wk_t, bk_t, kh)
            for st in range(4):
                v_proj(st)
            qk_proj(0, 0, xq_t, wq_t, bq_t, qh)

            # chunk (0,0), 2 fillers/j from j0: kh[0][g] lands before j=4g,
            # vh[st] at least 2 slots before j=st; qh[0][1] at the end
            # fillers ordered by DMA arrival (== deadline order): kh[0][g]
            # completes by slot 2g+3 <= scores deadline j=4g; vh[st] lands
            # before attnV(st) (emitted at iteration st+1); qh[0][1] last
            # (xq[1] arrives mid-chunk). j0-1 kept clean: they only need
            # sc0 data, so the exp stream starts as early as possible.
            f00 = qf(0, 1, xk_t, wk_t, bk_t, kh)
            f00 += [vf(st) for st in range(4, 8)]
            f00 += qf(0, 2, xk_t, wk_t, bk_t, kh)
            f00 += [vf(st) for st in range(8, 12)]
            f00 += qf(0, 3, xk_t, wk_t, bk_t, kh)
            f00 += [vf(st) for st in range(12, 16)]
            f00 += qf(0, 1, xq_t, wq_t, bq_t, qh)
            nrm = attn_chunk(0, 0, fillers=f00, per_j=2, fill_from=2)

            nrm = attn_chunk(0, 1, pre=nrm, fillers=(
                qf(0, 2, xq_t, wq_t, bq_t, qh)
                + qf(1, 0, xk_t, wk_t, bk_t, kh)
                + qf(1, 1, xk_t, wk_t, bk_t, kh)))
            nrm = attn_chunk(0, 2, pre=nrm, fillers=(
                qf(0, 3, xq_t, wq_t, bq_t, qh)
                + qf(1, 2, xk_t, wk_t, bk_t, kh)
                + qf(1, 3, xk_t, wk_t, bk_t, kh)))
            nrm = attn_chunk(0, 3, pre=nrm, fillers=(
                qf(1, 0, xq_t, wq_t, bq_t, qh)
                + qf(1, 1, xq_t, wq_t, bq_t, qh)
                + qf(1, 2, xq_t, wq_t, bq_t, qh)
                + qf(1, 3, xq_t, wq_t, bq_t, qh)))
            for ic in range(NSC):
                fill = []
                if ic > 0:
                    fill = [(lambda a, b, c: lambda: out_unit(a, b, c))
                            (ic - 1, it4, mc)
                            for it4 in range(4) for mc in range(2)]
                nrm = attn_chunk(1, ic, pre=nrm, fillers=fill)
            nrm()
            for it4 in range(4):
                for mc in range(2):
                    if (it4 + mc) % 2:
                        out_unit(3, it4, mc,
                                 cp=lambda o, p: nc.scalar.copy(o[:], p[:]))
                    else:
                        out_unit(3, it4, mc)

    nc.compile()
    return nc


def _prep_inputs(q, k, v, Wq, bq, Wk, bk, Wv, bv, Wo, bo):
    import ml_dtypes
    f = np.float32
    bf = ml_dtypes.bfloat16
    xT = {}
    for g in range(DP):
        for nm, a in (("q", q), ("k", k), ("v", v)):
            t = np.asarray(a[g], f).T.astype(bf)            # [1024, 2048]
            # [sc, p, dc*512+j] = t[dc*128+p, sc*512+j]
            t = t.reshape(NDC, 128, NSC, 512).transpose(2, 1, 0, 3)
            xT[(nm, g)] = np.ascontiguousarray(t.reshape(NSC, 128, NDC * 512))

    def packw(WT):   # [1024, 256] -> [128, 8*256]
        return np.ascontiguousarray(
            WT.reshape(NDC, 128, OPC).transpose(1, 0, 2).reshape(128, NDC * OPC))

    Wq, Wk, Wv, Wo = (np.asarray(a, f) for a in (Wq, Wk, Wv, Wo))
    bq, bk = (np.asarray(a, f) for a in (bq, bk))
    in_maps = []
    for c in range(NCORES):
        g, r = divmod(c, TP)
        sl = slice(r * OPC, (r + 1) * OPC)
        in_maps.append({
            "xqt": xT[("q", g)], "xkt": xT[("k", g)], "xvt": xT[("v", g)],
            "wqt": packw(Wq[sl].T.astype(bf)),
            "wkt": packw(Wk[sl].T.astype(bf)),
            "wvt": packw(Wv[sl].T.astype(bf)),
            "bq": bq[sl].reshape(2, 128, 1),
            "bk": bk[sl].reshape(2, 128, 1),
            "wot": np.ascontiguousarray(Wo[:, sl].T.astype(bf)).reshape(2, 128, D),
        })
    return in_maps


def kernel(q, k, v, Wq, bq, Wk, bk, Wv, bv, Wo, bo, _trace=False):
    from concourse.bass_utils import run_bass_kernel_spmd

    if "nc" not in _cache:
        _cache["nc"] = _build()
    nc = _cache["nc"]
    in_maps = _prep_inputs(q, k, v, Wq, bq, Wk, bk, Wv, bv, Wo, bo)
    res = run_bass_kernel_spmd(nc, in_maps, list(range(NCORES)), trace=_trace)
    _cache["last_exec_time_ns"] = res.exec_time_ns
    _cache["last_res"] = res
    parts = [res.results[c]["out"] for c in range(NCORES)]
    bo = np.asarray(bo, np.float32)
    bv = np.asarray(bv, np.float32)
    Wo = np.asarray(Wo, np.float32)
    bias = bo + bv @ Wo.T
    out = np.empty((B, S, D), np.float32)
    for g in range(DP):
        acc = parts[g * TP].astype(np.float32)
        for r in range(1, TP):
            acc = acc + parts[g * TP + r]
        out[g] = acc + bias
    return out
